# revision 10
# baseline (speedup 1.0000x reference)
"""Trainium2 Bass kernel for nn_BoundaryAttentionHead (gnn_message_passing).

reference computation:
    diff = (x[src] - x[dst])**2                    # [E, C]
    variance = scatter_add(diff, dst) / k          # [N, C]
    h = relu(variance @ W1 + b1)                   # [N, 64]
    out = sigmoid(h @ W2 + b2)                     # [N, 1]

Sharding: nodes across 8 cores (dst-segment partition); MLP weights
replicated; W1 pre-scaled by 1/k on host.

Per-node expansion (avoids per-edge subtraction):
    sum_j (x_sj - x_n)^2 = S2 - 2 x_n . S1 + deg * x_n^2
    S1 = sum_j x_sj,  S2 = sum_j x_sj^2

"bal" mode: combined fp16 tables [x | x^2] (512B rows, 4 overlapping
32768-row windows of x; int16 gather indices force the windowing).
Neighbours are balanced across windows (chain greedy over the window
overlaps) into A=2 aligned slots per window per node; remainder edges
are pooled densely per (group, window) and routed to their node with
one-hot (Sel) matmuls on PE.  One SWDGE dma_gather call per (group,
window) fetches S1/S2 contributions in one 512B descriptor per edge.

"indirect" mode (BAH_MODE=indirect): simple fallback, one indirect DMA
per (tile, slot); much slower but trivially correct.
"""
import os
import sys
import types

import numpy as np

_KERNEL_CACHE = {}
MODE = os.environ.get("BAH_MODE", "bal")

P = 128
NW = 4          # windows / gather tables
ACAP = 2        # aligned slots per window per node
WIN = 32768
GT = 7          # tiles per group
PAD_TGT = 1 << 20  # ovt target for pad entries (matches nothing)


def _install_ntff_hook():
    if "antenv.axon_hooks" in sys.modules:
        return
    sys.path.insert(0, "/root/.axon_site")
    try:
        from trn_agent_boot.trn_boot import _ntff_profile_via_ctypes
    except Exception:
        return
    mod = types.ModuleType("antenv.axon_hooks")
    _hook = [_ntff_profile_via_ctypes("/opt/axon/libaxon_pjrt.so")]
    mod.get_axon_ntff_profile_hook = lambda: _hook[0]
    mod.set_axon_ntff_profile_hook = lambda h: _hook.__setitem__(0, h)
    sys.modules["antenv.axon_hooks"] = mod


# ---------------------------------------------------------------- host side


def _node_lists(x, src, dst, k):
    """Per-node neighbour lists (-1 padded), degrees, per-core node ranges."""
    N, C = x.shape
    E = src.shape[0]
    n_cores = 8
    base = N // n_cores
    rem = N % n_cores
    starts = [c * base + min(c, rem) for c in range(n_cores)] + [N]

    src = np.asarray(src)
    dst = np.asarray(dst)

    fast = False
    if E % N == 0 and E // N > 0:
        K = E // N
        fast = np.array_equal(dst, np.repeat(np.arange(N, dtype=dst.dtype), K))

    if fast:
        nbrs = src.reshape(N, E // N).astype(np.int64)
        deg = np.full(N, E // N, dtype=np.int64)
    else:
        order = np.argsort(dst, kind="stable")
        ds = dst[order].astype(np.int64)
        ss = src[order].astype(np.int64)
        deg = np.bincount(ds, minlength=N)
        Kmax = int(deg.max()) if E else 1
        nbrs = np.full((N, Kmax), -1, dtype=np.int64)
        seg_off = np.zeros(N + 1, dtype=np.int64)
        np.cumsum(deg, out=seg_off[1:])
        pos = np.arange(E, dtype=np.int64) - seg_off[ds]
        nbrs[ds, pos] = ss
    return starts, nbrs, deg


def _window_starts(N):
    return np.array([round(w * (N - WIN + 1) / (NW - 1)) for w in range(NW)],
                    dtype=np.int64)


def _assign_core(nb, S):
    """Chain-greedy balanced window assignment for one core.

    nb: [n, K] neighbour src ids (-1 = pad).
    Returns aligned [n, NW, ACAP] table idx (0 = hole) and overflow
    (ov_w, ov_node, ov_idx) arrays (per-edge window / node / table idx).
    """
    n, K = nb.shape
    valid = nb >= 0
    s = np.where(valid, nb, 0)
    w_hi = np.searchsorted(S, s.ravel(), side="right").reshape(n, K) - 1
    w_lo = np.searchsorted(S + (WIN - 2), s.ravel(), side="left").reshape(n, K)
    assert (w_lo[valid] <= w_hi[valid]).all()

    aligned = np.zeros((n, NW, ACAP), dtype=np.int16)
    ov_w_l, ov_node_l, ov_idx_l = [], [], []

    take_prev = np.zeros(n, dtype=np.int64)
    rflex_prev = np.zeros((n, K), dtype=np.int64)
    flexmask_prev = np.zeros((n, K), dtype=bool)
    d_prev = np.zeros(n, dtype=np.int64)

    for w in range(NW):
        excl = valid & (w_lo == w) & (w_hi == w)
        flex = valid & (w_lo == w) & (w_hi == w + 1)
        r_excl = np.cumsum(excl, axis=1) - 1
        r_flex = np.cumsum(flex, axis=1) - 1
        E_w = excl.sum(axis=1)
        F_w = flex.sum(axis=1)

        deferred = flexmask_prev & (rflex_prev >= take_prev[:, None])
        pos_def = E_w[:, None] + (rflex_prev - take_prev[:, None])

        M_w = E_w + d_prev
        al_mand = np.minimum(M_w, ACAP)
        room = ACAP - al_mand
        take_w = np.minimum(F_w, room)

        idxval = (s - S[w] + 1).astype(np.int64)

        # aligned: mandatory (excl pos<ACAP, deferred pos<ACAP), then flex
        for mask, pos in ((excl, r_excl), (deferred, pos_def)):
            am = mask & (pos < ACAP)
            ii, jj = np.nonzero(am)
            aligned[ii, w, pos[am]] = idxval[am].astype(np.int16)
        fm = flex & (r_flex < take_w[:, None])
        ii, jj = np.nonzero(fm)
        aligned[ii, w, (al_mand[:, None] + r_flex)[fm]] = idxval[fm].astype(
            np.int16
        )

        # overflow: mandatory with pos >= ACAP
        for mask, pos in ((excl, r_excl), (deferred, pos_def)):
            om = mask & (pos >= ACAP)
            ii, jj = np.nonzero(om)
            if len(ii):
                ov_w_l.append(np.full(len(ii), w, dtype=np.int64))
                ov_node_l.append(ii)
                ov_idx_l.append(idxval[om])

        take_prev = take_w
        rflex_prev = r_flex
        flexmask_prev = flex
        d_prev = F_w - take_w

    assert (d_prev == 0).all()
    if ov_w_l:
        ov_w = np.concatenate(ov_w_l)
        ov_node = np.concatenate(ov_node_l)
        ov_idx = np.concatenate(ov_idx_l)
    else:
        ov_w = ov_node = ov_idx = np.zeros(0, dtype=np.int64)
    return aligned, ov_w, ov_node, ov_idx


def _plan_all(x, src, dst, k):
    N, C = x.shape
    starts, nbrs, deg = _node_lists(x, src, dst, k)
    S = _window_starts(N)
    n_cores = 8
    per_core = max(starts[c + 1] - starts[c] for c in range(n_cores))
    NT = (per_core + P - 1) // P
    n_nodes_pad = NT * P
    ngrp = (NT + GT - 1) // GT

    cores = []
    for c in range(n_cores):
        lo, hi = starts[c], starts[c + 1]
        nb = np.full((n_nodes_pad, nbrs.shape[1]), -1, dtype=np.int64)
        nb[: hi - lo] = nbrs[lo:hi]
        cores.append(_assign_core(nb, S))

    # pooled overflow per (group, window): sort by (tile, partition)
    # per-core pools; novc = max cols over cores; pairs = union of spans.
    pools = {}  # (c, g, w) -> (idxvals sorted, tgts sorted)
    novc = np.zeros((ngrp, NW), dtype=np.int64)
    for c in range(n_cores):
        _, ov_w, ov_node, ov_idx = cores[c]
        g_of = ov_node // (P * GT)
        t_loc = (ov_node // P) % GT
        p_of = ov_node % P
        for g in range(ngrp):
            for w in range(NW):
                m = (g_of == g) & (ov_w == w)
                tl, pp, iv = t_loc[m], p_of[m], ov_idx[m]
                o = np.lexsort((pp, tl))
                pools[(c, g, w)] = (iv[o], tl[o] * P + pp[o])
                novc[g, w] = max(novc[g, w], (len(iv) + P - 1) // P)

    # group layout: per (g): for w: [GT*ACAP aligned cols | novc[g,w] ov cols]
    groups = []
    total_cols = 0
    total_ovcols = 0
    for g in range(ngrp):
        tiles = list(range(g * GT, min(NT, g * GT + GT)))
        ngt = len(tiles)
        col = 0
        wblocks = []
        acol = {}
        ocol = {}
        for w in range(NW):
            wstart = col
            for ti in range(ngt):
                acol[(ti, w)] = col
                col += ACAP
            ocol[w] = col
            col += int(novc[g, w])
            wblocks.append((wstart, col))
        # union (ovcol, tile) pairs across cores
        pairs = set()
        for w in range(NW):
            for c in range(n_cores):
                iv, tg = pools[(c, g, w)]
                for j in range(len(iv)):
                    pairs.add((ocol[w] + j // P, int(tg[j]) // P))
        pairs = sorted(pairs)
        groups.append(
            dict(tiles=tiles, ngt=ngt, ncol=col, wblocks=wblocks,
                 acol=acol, ocol=ocol, pairs=pairs,
                 novc=int(novc[g].sum()))
        )
        total_cols += col
        total_ovcols += int(novc[g].sum())

    # per-core packed idx16 / ovt
    core_data = []
    for c in range(n_cores):
        aligned, _, _, _ = cores[c]
        colvals = np.zeros((total_cols, P), dtype=np.int16)
        ovt = np.full((total_ovcols, P), float(PAD_TGT), dtype=np.float32)
        cb = 0
        ob = 0
        for g, gd in enumerate(groups):
            ngt = gd["ngt"]
            for w in range(NW):
                for ti in range(ngt):
                    t = gd["tiles"][ti]
                    blk = aligned[t * P : (t + 1) * P, w, :]  # [P, ACAP]
                    c0 = cb + gd["acol"][(ti, w)]
                    colvals[c0 : c0 + ACAP, :] = blk.T
                iv, tg = pools[(c, g, w)]
                o0 = cb + gd["ocol"][w]
                ncol_w = int(novc[g, w])
                o_glob = ob + sum(int(novc[g, ww]) for ww in range(w))
                if len(iv):
                    buf = np.zeros(ncol_w * P, dtype=np.int16)
                    tbuf = np.full(ncol_w * P, float(PAD_TGT), dtype=np.float32)
                    buf[: len(iv)] = iv.astype(np.int16)
                    tbuf[: len(iv)] = tg.astype(np.float32)
                    colvals[o0 : o0 + ncol_w, :] = buf.reshape(ncol_w, P)
                    ovt[o_glob : o_glob + ncol_w, :] = tbuf.reshape(ncol_w, P)
            cb += gd["ncol"]
            ob += gd["novc"]
        # host-built one-hot Sel blocks, one [P, P] per (ovcol, tile) pair
        total_pairs = sum(len(gd["pairs"]) for gd in groups)
        seld = np.zeros((P, max(total_pairs, 1) * P), dtype=np.float16)
        pi = 0
        ob = 0
        pcol = np.arange(P)
        for g, gd in enumerate(groups):
            for (col, t) in gd["pairs"]:
                for w in range(NW):
                    o0 = gd["ocol"][w]
                    we = gd["wblocks"][w][1]
                    if o0 <= col < we:
                        row = (ob + sum(int(novc[g, ww]) for ww in range(w))
                               + (col - o0))
                        break
                tgt = ovt[row, :]
                seld[:, pi * P : (pi + 1) * P] = (
                    tgt[:, None] == (t * P + pcol)[None, :]
                ).astype(np.float16)
                pi += 1
            ob += gd["novc"]
        # wrap: flat element i of a col -> (i%16, i//16), replicated x8
        v = colvals.reshape(total_cols, 8, 16)
        idx16 = np.tile(
            np.ascontiguousarray(v.transpose(2, 0, 1)).reshape(
                16, total_cols * 8
            ),
            (8, 1),
        )
        core_data.append(
            dict(idx16=idx16, ovt=np.ascontiguousarray(ovt.T), seld=seld)
        )

    return dict(
        N=N, C=C, NT=NT, n_nodes_pad=n_nodes_pad, ngrp=ngrp, S=S,
        starts=starts, deg=deg, groups=groups,
        total_cols=total_cols, total_ovcols=total_ovcols,
        core_data=core_data,
    )


def _build_tables(x):
    """NW combined fp16 tables [WIN, 2C]: row j+1 = [x[S+j] | x[S+j]^2]."""
    N, C = x.shape
    S = _window_starts(N)
    xsq = (x.astype(np.float64) ** 2).astype(np.float16)
    x16 = x.astype(np.float16)
    tabs = []
    for w in range(NW):
        t = np.zeros((WIN, 2 * C), dtype=np.float16)
        lo = int(S[w])
        hi = min(N, lo + WIN - 1)
        t[1 : 1 + hi - lo, :C] = x16[lo:hi]
        t[1 : 1 + hi - lo, C:] = xsq[lo:hi]
        tabs.append(t)
    return tabs


# ------------------------------------------------------------- device side


def _build_bal(plan, H):
    import concourse.bacc as bacc
    import concourse.tile as tile
    from concourse import mybir
    from concourse.library_config import mlp
    from concourse.masks import make_identity

    F32 = mybir.dt.float32
    F16 = mybir.dt.float16
    I16 = mybir.dt.int16
    C = plan["C"]
    C2 = 2 * C
    groups = plan["groups"]
    n_nodes_pad = plan["n_nodes_pad"]

    nc = bacc.Bacc("TRN2", num_swdge_queues=4, dynamic_dma_scratch_size=65536)
    xq_t = [
        nc.dram_tensor(f"xq{w}", [WIN, C2], F16, kind="ExternalInput")
        for w in range(NW)
    ]
    idx16 = nc.dram_tensor(
        "idx16", [P, plan["total_cols"] * 8], I16, kind="ExternalInput"
    )
    xls = nc.dram_tensor("xls", [n_nodes_pad, 2 * C], F16, kind="ExternalInput")
    total_pairs = sum(len(g["pairs"]) for g in groups)
    seld = nc.dram_tensor(
        "seld", [P, max(total_pairs, 1) * P], F16, kind="ExternalInput"
    )
    w1k = nc.dram_tensor("w1k", [C, H], F16, kind="ExternalInput")
    b1 = nc.dram_tensor("b1", [H, 1], F32, kind="ExternalInput")
    w2 = nc.dram_tensor("w2", [H, 1], F32, kind="ExternalInput")
    b2 = nc.dram_tensor("b2", [1, 1], F32, kind="ExternalInput")
    y = nc.dram_tensor("y", [1, n_nodes_pad], F32, kind="ExternalOutput")

    qrr = [0]

    with tile.TileContext(nc) as tc:
        with tc.tile_critical():
            nc.gpsimd.load_library(mlp)
        with (
            tc.tile_pool(name="const", bufs=1) as cpool,
            tc.tile_pool(name="grp", bufs=2) as gpool,
            tc.tile_pool(name="selp", bufs=2) as selp,
            tc.tile_pool(name="sbuf", bufs=2) as pool,
            tc.tile_pool(name="tpool", bufs=1) as tpool,
            tc.tile_pool(name="spool", bufs=2) as spool,
            tc.tile_pool(name="hbuf", bufs=1) as hpool,
            tc.tile_pool(name="psum", bufs=2, space="PSUM") as psum,
            tc.tile_pool(name="ovp", bufs=2, space="PSUM") as ovpool,
            tc.tile_pool(name="opsum", bufs=1, space="PSUM") as opsum,
        ):
            ident = cpool.tile([P, P], F32)
            make_identity(nc, ident[:])
            w1k_t = cpool.tile([C, H], F16)
            nc.sync.dma_start(out=w1k_t[:], in_=w1k[:])
            b1_t = cpool.tile([H, 1], F32)
            nc.sync.dma_start(out=b1_t[:], in_=b1[:])
            w2_t = cpool.tile([H, 1], F32)
            nc.sync.dma_start(out=w2_t[:], in_=w2[:])
            b2_t = cpool.tile([1, 1], F32)
            nc.sync.dma_start(out=b2_t[:], in_=b2[:])

            cum_col = [0]
            cum_pair = [0]
            for g in groups:
                cum_col.append(cum_col[-1] + g["ncol"])
                cum_pair.append(cum_pair[-1] + len(g["pairs"]))

            ctxs = {}

            def emit_gathers(gi):
                g = groups[gi]
                ncol = g["ncol"]
                novc = g["novc"]
                ctx = {}
                grp = gpool.tile([P, ncol * C2], F16, tag="grp")
                grpv = grp[:].rearrange("p (t c) -> p t c", c=C2)
                idxg = pool.tile([P, ncol * 8], I16, tag="idxg")
                nc.scalar.dma_start(
                    out=idxg[:],
                    in_=idx16[:, cum_col[gi] * 8 : (cum_col[gi] + ncol) * 8],
                )
                for w in range(NW):
                    ws, we = g["wblocks"][w]
                    n = (we - ws) * P
                    nc.gpsimd.dma_gather(
                        grpv[:, ws:we, :],
                        xq_t[w][:],
                        idxg[:, ws * 8 : we * 8],
                        n, n, C2,
                        queue_num=qrr[0] % 4,
                        single_packet=(we - ws) <= 8,
                    )
                    qrr[0] += 1
                ctx["grp"] = grp
                ctx["grpv"] = grpv
                pairs = g["pairs"]
                if pairs:
                    sel_g = selp.tile([P, len(pairs) * P], F16, tag="sel_g")
                    nc.sync.dma_start(
                        out=sel_g[:],
                        in_=seld[
                            :,
                            cum_pair[gi] * P : (cum_pair[gi] + len(pairs)) * P,
                        ],
                    )
                    ctx["sel_g"] = sel_g
                ctxs[gi] = ctx

            def emit_compute(gi):
                g = groups[gi]
                ngt = g["ngt"]
                pairs = g["pairs"]
                ctx = ctxs.pop(gi)
                grp = ctx["grp"]
                grpv = ctx["grpv"]

                h_g = hpool.tile([H, GT * P], F32, tag="h_g")
                o_ps = opsum.tile([1, GT * P], F32, space="PSUM", tag="o_ps")
                y_sb = hpool.tile([1, GT * P], F32, tag="y_sb")

                # batched aligned tree: chain-sum the 4 window blocks (fp16)
                AW = ngt * ACAP * C2
                a0 = [g["acol"][(0, w)] * C2 for w in range(NW)]
                t1 = tpool.tile([P, AW], F16, tag="t1")
                nc.vector.tensor_add(
                    out=t1[:],
                    in0=grp[:, a0[0] : a0[0] + AW],
                    in1=grp[:, a0[1] : a0[1] + AW],
                )
                for w in (2, 3):
                    nc.vector.tensor_add(
                        out=t1[:], in0=t1[:], in1=grp[:, a0[w] : a0[w] + AW]
                    )
                s12_all = spool.tile([P, ngt * C2], F32, tag="s12")
                t1v = t1[:].rearrange("p (t a c) -> p t a c", a=ACAP, c=C2)
                nc.vector.tensor_add(
                    out=s12_all[:].rearrange("p (t c) -> p t c", c=C2),
                    in0=t1v[:, :, 0, :],
                    in1=t1v[:, :, 1, :],
                )

                # overflow routing into s12_all
                for ti in range(ngt):
                    tp = [i for i, (col, tt) in enumerate(pairs) if tt == ti]
                    if not tp:
                        continue
                    sel_g = ctx["sel_g"]
                    ovps = ovpool.tile([P, C2], F32, space="PSUM", tag="ovps")
                    for mi, li in enumerate(tp):
                        col = pairs[li][0]
                        nc.tensor.matmul(
                            out=ovps[:],
                            lhsT=sel_g[:, li * P : (li + 1) * P],
                            rhs=grpv[:, col, :],
                            start=(mi == 0),
                            stop=(mi == len(tp) - 1),
                        )
                    sl = s12_all[:, ti * C2 : (ti + 1) * C2]
                    nc.vector.tensor_add(out=sl, in0=sl, in1=ovps[:])

                # group-batched V = S2 - 2 xl . S1 + (sdx)^2
                g0 = g["tiles"][0]
                xlg = pool.tile([P, ngt * 2 * C], F16, tag="xlg")
                nc.scalar.dma_start(
                    out=xlg[:].rearrange("p (t c) -> p t c", c=2 * C),
                    in_=xls[g0 * P : (g0 + ngt) * P, :].rearrange(
                        "(t p) c -> p t c", p=P
                    ),
                )
                xlgv = xlg[:].rearrange("p (t c) -> p t c", c=2 * C)
                s12v = s12_all[:].rearrange("p (t c) -> p t c", c=C2)
                xlm2 = tpool.tile([P, ngt * C], F32, tag="xlm2")
                nc.scalar.mul(
                    out=xlm2[:].rearrange("p (t c) -> p t c", c=C),
                    in_=xlgv[:, :, 0:C],
                    mul=-2.0,
                )
                u_all = tpool.tile([P, ngt * C], F32, tag="u_all")
                nc.scalar.activation(
                    u_all[:].rearrange("p (t c) -> p t c", c=C),
                    xlgv[:, :, C : 2 * C],
                    mybir.ActivationFunctionType.Square,
                )
                v_all = tpool.tile([P, ngt * C], F32, tag="v_all")
                nc.vector.tensor_tensor(
                    out=v_all[:].rearrange("p (t c) -> p t c", c=C),
                    in0=xlm2[:].rearrange("p (t c) -> p t c", c=C),
                    in1=s12v[:, :, 0:C],
                    op=mybir.AluOpType.mult,
                )
                nc.vector.tensor_add(
                    out=v_all[:].rearrange("p (t c) -> p t c", c=C),
                    in0=v_all[:].rearrange("p (t c) -> p t c", c=C),
                    in1=s12v[:, :, C:C2],
                )
                nc.vector.tensor_add(
                    out=v_all[:], in0=v_all[:], in1=u_all[:]
                )

                for ti in range(ngt):
                    vt_ps = psum.tile([C, P], F32, space="PSUM", tag="vt")
                    nc.tensor.transpose(
                        out=vt_ps[:],
                        in_=v_all[:, ti * C : (ti + 1) * C],
                        identity=ident[:],
                    )
                    vt = pool.tile([C, P], F16, tag="vts")
                    nc.scalar.copy(out=vt[:], in_=vt_ps[:])
                    h_ps = psum.tile([H, P], F32, space="PSUM", tag="h_ps")
                    nc.tensor.matmul(
                        out=h_ps[:], lhsT=w1k_t[:], rhs=vt[:],
                        start=True, stop=True,
                    )
                    nc.scalar.activation(
                        h_g[:, ti * P : (ti + 1) * P],
                        h_ps[:],
                        mybir.ActivationFunctionType.Relu,
                        bias=b1_t[:, :1],
                    )

                for s in range(0, ngt * P, 512):
                    e = min(s + 512, ngt * P)
                    nc.tensor.matmul(
                        out=o_ps[:, s:e], lhsT=w2_t[:], rhs=h_g[:, s:e],
                        start=True, stop=True,
                    )
                nc.scalar.activation(
                    y_sb[:, : ngt * P],
                    o_ps[:, : ngt * P],
                    mybir.ActivationFunctionType.Sigmoid,
                    bias=b2_t[:, :1],
                )
                g0 = g["tiles"][0]
                nc.sync.dma_start(
                    out=y[:, g0 * P : g0 * P + ngt * P], in_=y_sb[:, : ngt * P]
                )

            LOOKAHEAD = 1
            for gi in range(len(groups) + LOOKAHEAD):
                if gi < len(groups):
                    emit_gathers(gi)
                if gi >= LOOKAHEAD:
                    emit_compute(gi - LOOKAHEAD)
    nc.compile()
    return nc


# ------------------------------------------------------- v1 fallback build


def _build_indirect(N, C, KS, NT, n_nodes_pad, H):
    import concourse.bass as bass
    import concourse.bacc as bacc
    import concourse.tile as tile
    from concourse import mybir
    from concourse.masks import make_identity

    F32 = mybir.dt.float32
    I32 = mybir.dt.int32
    OG = 8

    nc = bacc.Bacc("TRN2")
    x = nc.dram_tensor("x", [N, C], F32, kind="ExternalInput")
    idx = nc.dram_tensor("idx", [n_nodes_pad, KS], I32, kind="ExternalInput")
    w1k = nc.dram_tensor("w1k", [C, H], F32, kind="ExternalInput")
    b1 = nc.dram_tensor("b1", [H, 1], F32, kind="ExternalInput")
    w2 = nc.dram_tensor("w2", [H, 1], F32, kind="ExternalInput")
    b2 = nc.dram_tensor("b2", [1, 1], F32, kind="ExternalInput")
    y = nc.dram_tensor("y", [1, n_nodes_pad], F32, kind="ExternalOutput")

    with tile.TileContext(nc) as tc:
        with (
            tc.tile_pool(name="const", bufs=1) as cpool,
            tc.tile_pool(name="sbuf", bufs=2) as pool,
            tc.tile_pool(name="hbuf", bufs=2) as hpool,
            tc.tile_pool(name="psum", bufs=2, space="PSUM") as psum,
            tc.tile_pool(name="opsum", bufs=1, space="PSUM") as opsum,
        ):
            ident = cpool.tile([P, P], F32)
            make_identity(nc, ident[:])
            w1k_t = cpool.tile([C, H], F32)
            nc.sync.dma_start(out=w1k_t[:], in_=w1k[:])
            b1_t = cpool.tile([H, 1], F32)
            nc.sync.dma_start(out=b1_t[:], in_=b1[:])
            w2_t = cpool.tile([H, 1], F32)
            nc.sync.dma_start(out=w2_t[:], in_=w2[:])
            b2_t = cpool.tile([1, 1], F32)
            nc.sync.dma_start(out=b2_t[:], in_=b2[:])

            for g in range(0, NT, OG):
                ng = min(OG, NT - g)
                h_g = hpool.tile([H, OG * P], F32, tag="h_g")
                o_ps = opsum.tile([1, OG * P], F32, space="PSUM", tag="o_ps")
                y_sb = hpool.tile([1, OG * P], F32, tag="y_sb")
                for ti in range(ng):
                    t = g + ti
                    idx_t = pool.tile([P, KS], I32, tag="idx")
                    nc.sync.dma_start(out=idx_t[:], in_=idx[t * P : (t + 1) * P, :])
                    xs = pool.tile([P, KS * C], F32, tag="xs")
                    for j in range(KS):
                        nc.gpsimd.indirect_dma_start(
                            out=xs[:, j * C : (j + 1) * C],
                            out_offset=None,
                            in_=x[:],
                            in_offset=bass.IndirectOffsetOnAxis(
                                ap=idx_t[:, j : j + 1], axis=0
                            ),
                        )
                    xd_b = xs[:, (KS - 1) * C : KS * C][:, None, :].to_broadcast(
                        [P, KS, C]
                    )
                    nc.vector.tensor_tensor(
                        out=xs[:], in0=xs[:], in1=xd_b,
                        op=mybir.AluOpType.subtract,
                    )
                    nc.scalar.activation(
                        xs[:], xs[:], mybir.ActivationFunctionType.Square
                    )
                    v = pool.tile([P, C], F32, tag="v")
                    nc.vector.reduce_sum(
                        out=v[:],
                        in_=xs[:].rearrange("p (j c) -> p c j", j=KS),
                        axis=mybir.AxisListType.X,
                    )
                    vt_ps = psum.tile([C, P], F32, space="PSUM", tag="vt")
                    nc.tensor.transpose(out=vt_ps[:], in_=v[:], identity=ident[:])
                    vt = pool.tile([C, P], F32, tag="vts")
                    nc.vector.tensor_copy(out=vt[:], in_=vt_ps[:])
                    h_ps = psum.tile([H, P], F32, space="PSUM", tag="h_ps")
                    nc.tensor.matmul(
                        out=h_ps[:], lhsT=w1k_t[:], rhs=vt[:], start=True, stop=True
                    )
                    nc.scalar.activation(
                        h_g[:, ti * P : (ti + 1) * P],
                        h_ps[:],
                        mybir.ActivationFunctionType.Relu,
                        bias=b1_t[:, :1],
                    )
                for s in range(0, ng * P, 512):
                    e = min(s + 512, ng * P)
                    nc.tensor.matmul(
                        out=o_ps[:, s:e], lhsT=w2_t[:], rhs=h_g[:, s:e],
                        start=True, stop=True,
                    )
                nc.scalar.activation(
                    y_sb[:, : ng * P],
                    o_ps[:, : ng * P],
                    mybir.ActivationFunctionType.Sigmoid,
                    bias=b2_t[:, :1],
                )
                nc.sync.dma_start(
                    out=y[:, g * P : g * P + ng * P], in_=y_sb[:, : ng * P]
                )
    nc.compile()
    return nc


# ------------------------------------------------------------------ driver


def _mlp_consts(W1, b1, W2, b2, k, H, w1_f16=False):
    kk = float(np.asarray(k))
    w1 = np.asarray(W1, dtype=np.float32) / kk
    if w1_f16:
        w1 = w1.astype(np.float16)
    return (
        np.ascontiguousarray(w1),
        np.ascontiguousarray(np.asarray(b1, dtype=np.float32).reshape(H, 1)),
        np.ascontiguousarray(np.asarray(W2, dtype=np.float32).reshape(H, 1)),
        np.ascontiguousarray(np.asarray(b2, dtype=np.float32).reshape(1, 1)),
    )


def _run_indirect(x, src, dst, k, W1, b1, W2, b2):
    from concourse.bass_utils import run_bass_kernel_spmd

    N, C = x.shape
    H = W1.shape[1]
    starts, nbrs, deg = _node_lists(x, np.asarray(src), np.asarray(dst), k)
    K = nbrs.shape[1]
    KS = K + 1
    n_cores = 8
    per_core = max(starts[c + 1] - starts[c] for c in range(n_cores))
    NT = (per_core + P - 1) // P
    n_nodes_pad = NT * P

    key = ("ind", N, C, KS, NT, n_nodes_pad, H)
    if key not in _KERNEL_CACHE:
        _KERNEL_CACHE[key] = _build_indirect(N, C, KS, NT, n_nodes_pad, H)
    nc = _KERNEL_CACHE[key]

    w1k, b1v, w2v, b2v = _mlp_consts(W1, b1, W2, b2, k, H)

    in_maps = []
    for c in range(n_cores):
        lo, hi = starts[c], starts[c + 1]
        idx = np.zeros((n_nodes_pad, KS), dtype=np.int32)
        nb = nbrs[lo:hi]
        own = np.arange(lo, hi, dtype=np.int64)
        nb2 = np.where(nb >= 0, nb, own[:, None])
        idx[: hi - lo, :K] = nb2
        idx[: hi - lo, K] = own
        in_maps.append(
            {"x": x, "idx": idx, "w1k": w1k, "b1": b1v, "w2": w2v, "b2": b2v}
        )

    res = run_bass_kernel_spmd(nc, in_maps, core_ids=list(range(n_cores)))
    out = np.empty((N, 1), dtype=np.float32)
    for c in range(n_cores):
        lo, hi = starts[c], starts[c + 1]
        out[lo:hi, 0] = res.results[c]["y"][0, : hi - lo]
    return out


def _run_bal(x, src, dst, k, W1, b1, W2, b2):
    from concourse.bass_utils import run_bass_kernel_spmd

    N, C = x.shape
    H = W1.shape[1]
    plan = _plan_all(x, np.asarray(src), np.asarray(dst), k)
    n_cores = 8
    n_nodes_pad = plan["n_nodes_pad"]
    starts = plan["starts"]

    key = ("bal", N, C, plan["NT"], n_nodes_pad, H, plan["total_cols"],
           plan["total_ovcols"], 5,
           tuple(tuple(g["pairs"]) for g in plan["groups"]))
    if key not in _KERNEL_CACHE:
        _KERNEL_CACHE[key] = _build_bal(plan, H)
    nc = _KERNEL_CACHE[key]

    w1k, b1v, w2v, b2v = _mlp_consts(W1, b1, W2, b2, k, H, w1_f16=True)
    tabs = _build_tables(x)

    in_maps = []
    for c in range(n_cores):
        lo, hi = starts[c], starts[c + 1]
        xl = np.zeros((n_nodes_pad, 2 * C), dtype=np.float16)
        xl[: hi - lo, :C] = x[lo:hi]
        xl[: hi - lo, C:] = x[lo:hi] * np.sqrt(
            plan["deg"][lo:hi].astype(np.float32)
        )[:, None]
        m = {
            "idx16": plan["core_data"][c]["idx16"],
            "seld": plan["core_data"][c]["seld"],
            "xls": xl,
            "w1k": w1k,
            "b1": b1v,
            "w2": w2v,
            "b2": b2v,
        }
        for w in range(NW):
            m[f"xq{w}"] = tabs[w]
        in_maps.append(m)

    res = run_bass_kernel_spmd(nc, in_maps, core_ids=list(range(n_cores)))
    out = np.empty((N, 1), dtype=np.float32)
    for c in range(n_cores):
        lo, hi = starts[c], starts[c + 1]
        out[lo:hi, 0] = res.results[c]["y"][0, : hi - lo]
    return out


def kernel(x, src, dst, k, W1, b1, W2, b2):
    _install_ntff_hook()
    x = np.ascontiguousarray(np.asarray(x, dtype=np.float32))
    N = x.shape[0]
    if (
        MODE == "bal"
        and x.shape[1] == 128
        and WIN < N <= (NW - 1) * (WIN - 1) + WIN
    ):
        return _run_bal(x, src, dst, k, W1, b1, W2, b2)
    return _run_indirect(x, src, dst, k, W1, b1, W2, b2)


def run_traced(**inputs):
    """test.py helper: run with NTFF tracing, return (output, exec_time_ns)."""
    _install_ntff_hook()
    import concourse.bass_utils as bu

    orig = bu.run_bass_kernel_spmd
    holder = {}

    def wrapper(nc, in_maps, core_ids, **kw):
        kw["trace"] = True
        r = orig(nc, in_maps, core_ids, **kw)
        holder["exec_time_ns"] = r.exec_time_ns
        return r

    bu.run_bass_kernel_spmd = wrapper
    try:
        out = kernel(**inputs)
    finally:
        bu.run_bass_kernel_spmd = orig
    return out, holder.get("exec_time_ns")
